# revision 8
# baseline (speedup 1.0000x reference)
"""GsLstm (graph LSTM message passing) Trainium2 Bass kernel.

Problem: B=8 batch of independent graphs, N=2000 nodes, H=128 hidden, K=32
neighbors, 2 layers.  Per layer:
    in_agg  = sum_k in_mask[n,k]  * h[in_idx[n,k]]
    out_agg = sum_k out_mask[n,k] * h[out_idx[n,k]]
    pre[g]  = in_agg @ w_in[g] + out_agg @ w_out[g] + h @ (u_in[g]+u_out[g]) + b[g]
    i,o,f = sigmoid(pre[0..2]); g = tanh(pre[3])
    c = f*c + i*g ; h = o*tanh(c)

Sharding: one batch element per NeuronCore (8 cores).

Strategy per core: the masked gather-reduce is a sparse matmul agg = A @ h
with A[n,m] = sum_k mask[n,k]*[idx[n,k]==m].  Materializing the gather
(32MB/agg) through DMA or vector engines is far slower than the dense
matmul on the PE at 2.4GHz, so we build dense A^T on the host from the
(constant) indices/masks, ship it bf16, and compute
    agg^T[f,n] = sum_c h_chunk[mc,f]^T @ A^T[mc,n]
on the TensorEngine with PSUM accumulation over the 16 m-chunks.
Gate matmuls run in fp32r (1 cyc/row at free>=256, ~1e-4 rel err).
ACT applies bias+sigmoid/tanh straight out of PSUM; DVE does the LSTM
cell update; a DMA transpose re-lays h out for the next layer's lhsT.
"""

import functools

import numpy as np
import ml_dtypes

import bass_rust as _bass_rust
import concourse.bass as bass
import concourse.mybir as mybir
import concourse.tile as tile

B, N, H, K = 8, 2000, 128, 32
LAYERS = 2
NCORES = 8
CH = 128                      # m-chunk size (contraction tiling)
NCH = (N + CH - 1) // CH      # 16 chunks, last has 80 rows
F32 = mybir.dt.float32
BF16 = mybir.dt.bfloat16
F16 = mybir.dt.float16
F32R = mybir.dt.float32r
AF = mybir.ActivationFunctionType

# n-column tiling: halves for aggregation PSUM, quarters for gates/pointwise
HALF_COLS = [(0, 1024), (1024, N - 1024)]          # (offset, width)
QUARTERS = [(0, 512), (512, 512), (1024, 512), (1536, N - 1536)]
HALF_QUARTERS = [[(0, 512), (512, 512)], [(0, 512), (512, N - 1536)]]


def chunk_rows(c: int) -> int:
    return min(CH, N - c * CH)


def build_nc() -> bass.Bass:
    nc = bass.Bass("TRN2", target_bir_lowering=False, debug=False)

    # --- DRAM I/O ---------------------------------------------------------
    at_in_d = nc.dram_tensor("at_in", [128, NCH, N], F16, kind="ExternalInput").ap()
    at_out_d = nc.dram_tensor("at_out", [128, NCH, N], F16, kind="ExternalInput").ap()
    h0t_d = nc.dram_tensor("h0t", [128, N], F32R, kind="ExternalInput").ap()
    h0n_d = nc.dram_tensor("h0n", [128, NCH, CH], F16, kind="ExternalInput").ap()
    c0t_d = nc.dram_tensor("c0t", [128, N], F32, kind="ExternalInput").ap()
    w_d = nc.dram_tensor("w", [128, 12, H], F32R, kind="ExternalInput").ap()
    bt_d = nc.dram_tensor("bt", [128, 4], F32, kind="ExternalInput").ap()
    out_d = nc.dram_tensor("ht_out", [128, N], F32, kind="ExternalOutput").ap()

    with tile.TileContext(nc) as tc:
        with (
            tc.tile_pool(name="res", bufs=1) as res,
            tc.tile_pool(name="gate", bufs=1) as gatep,
            tc.tile_pool(name="pw", bufs=1) as pwp,
            tc.tile_pool(name="psA", bufs=1, space="PSUM") as psA,
            tc.tile_pool(name="psG", bufs=1, space="PSUM") as psG,
        ):
            # --- SBUF residents ------------------------------------------
            at_in = res.tile([128, NCH, N], F16)
            at_out = res.tile([128, NCH, N], F16)
            h_nat = res.tile([128, NCH, CH], F16)   # h, natural layout chunks (lhsT)
            ht = res.tile([128, N], F32R)            # h^T (gate rhs + final out)
            # h^T bf16 (DMA-transpose source; padded to 2048 — the xbar
            # needs 128-multiple source columns)
            ht_bf = res.tile([128, NCH * CH], F16)
            ct = res.tile([128, N], F32)             # c^T
            agg_in = res.tile([128, N], F32R)        # in_agg^T
            agg_out = res.tile([128, N], F32R)       # out_agg^T
            w_sb = res.tile([128, 12, H], F32R)      # u(4), w_in(4), w_out(4)
            bt_sb = res.tile([128, 4], F32)

            # --- input DMAs ----------------------------------------------
            nc.vector.memset(ht_bf[:, N:], 0.0)
            nc.sync.dma_start(w_sb[:], w_d[:])
            nc.sync.dma_start(bt_sb[:], bt_d[:])
            nc.sync.dma_start(ht[:], h0t_d[:])
            nc.sync.dma_start(ct[:], c0t_d[:])
            nc.sync.dma_start(h_nat[:], h0n_d[:])
            # A chunks, half-major so layer-1 half-0 aggregation can start
            # as soon as its 32 chunk-halves have landed.
            for hoff, hw in HALF_COLS:
                for c in range(NCH):
                    nc.sync.dma_start(
                        at_in[:, c, hoff:hoff + hw], at_in_d[:, c, hoff:hoff + hw]
                    )
                    nc.sync.dma_start(
                        at_out[:, c, hoff:hoff + hw], at_out_d[:, c, hoff:hoff + hw]
                    )

            # --- layers ---------------------------------------------------
            for layer in range(LAYERS):
                for hi, (hoff, hw) in enumerate(HALF_COLS):
                    # aggregation: accumulate over m-chunks into PSUM
                    p_in = psA.tile([128, 1024], F32, tag="paggin")
                    p_out = psA.tile([128, 1024], F32, tag="paggout")
                    for c in range(NCH):
                        mc = chunk_rows(c)
                        lhsT = h_nat[0:mc, c, :]
                        st, sp = c == 0, c == NCH - 1
                        for qoff, qw in HALF_QUARTERS[hi]:
                            nc.tensor.matmul(
                                p_in[:, qoff:qoff + qw],
                                lhsT=lhsT,
                                rhs=at_in[0:mc, c, hoff + qoff:hoff + qoff + qw],
                                start=st, stop=sp,
                            )
                            nc.tensor.matmul(
                                p_out[:, qoff:qoff + qw],
                                lhsT=lhsT,
                                rhs=at_out[0:mc, c, hoff + qoff:hoff + qoff + qw],
                                start=st, stop=sp,
                            )
                    # evict (and round to f32r); split across ACT and DVE
                    nc.scalar.activation(
                        agg_in[:, hoff:hoff + hw], p_in[:, 0:hw], AF.Copy
                    )
                    nc.vector.tensor_copy(agg_out[:, hoff:hoff + hw], p_out[:, 0:hw])

                    # gates + pointwise for the two quarters of this half
                    for qoff, qw in HALF_QUARTERS[hi]:
                        goff = hoff + qoff  # global column offset
                        pg = psG.tile([128, 4, 512], F32, tag="pg")
                        for g in range(4):
                            nc.tensor.matmul(
                                pg[:, g, 0:qw], lhsT=w_sb[:, g, :],
                                rhs=ht[:, goff:goff + qw], start=True, stop=False,
                            )
                            nc.tensor.matmul(
                                pg[:, g, 0:qw], lhsT=w_sb[:, 4 + g, :],
                                rhs=agg_in[:, goff:goff + qw], start=False, stop=False,
                            )
                            nc.tensor.matmul(
                                pg[:, g, 0:qw], lhsT=w_sb[:, 8 + g, :],
                                rhs=agg_out[:, goff:goff + qw], start=False, stop=True,
                            )
                        ig = gatep.tile([128, 512], F32, tag="ig")
                        og = gatep.tile([128, 512], F32, tag="og")
                        fg = gatep.tile([128, 512], F32, tag="fg")
                        gg = gatep.tile([128, 512], F32, tag="gg")
                        nc.scalar.activation(
                            ig[:, 0:qw], pg[:, 0, 0:qw], AF.Sigmoid, bias=bt_sb[:, 0:1]
                        )
                        nc.scalar.activation(
                            og[:, 0:qw], pg[:, 1, 0:qw], AF.Sigmoid, bias=bt_sb[:, 1:2]
                        )
                        nc.scalar.activation(
                            fg[:, 0:qw], pg[:, 2, 0:qw], AF.Sigmoid, bias=bt_sb[:, 2:3]
                        )
                        nc.scalar.activation(
                            gg[:, 0:qw], pg[:, 3, 0:qw], AF.Tanh, bias=bt_sb[:, 3:4]
                        )
                        cs = ct[:, goff:goff + qw]
                        t = pwp.tile([128, 512], F32, tag="t")
                        tc_b = pwp.tile([128, 512], F32, tag="tc")
                        nc.vector.tensor_mul(t[:, 0:qw], ig[:, 0:qw], gg[:, 0:qw])
                        nc.vector.tensor_mul(cs, fg[:, 0:qw], cs)
                        nc.vector.tensor_add(cs, cs, t[:, 0:qw])
                        nc.scalar.activation(tc_b[:, 0:qw], cs, AF.Tanh)
                        nc.vector.tensor_mul(
                            ht[:, goff:goff + qw], og[:, 0:qw], tc_b[:, 0:qw]
                        )
                        if layer < LAYERS - 1:
                            nc.scalar.activation(
                                ht_bf[:, goff:goff + qw], ht[:, goff:goff + qw],
                                AF.Copy,
                            )

                # re-layout h for next layer's lhsT via DMA transpose
                if layer < LAYERS - 1:
                    for c in range(NCH):
                        nc.sync.dma_start(
                            h_nat[:, c, :],
                            ht_bf[:, c * CH:(c + 1) * CH],
                            transpose=True,
                        )

            nc.sync.dma_start(out_d[:], ht[:].bitcast(F32))

    _bass_rust.generate_event_semaphores(nc)
    return nc


# ---------------------------------------------------------------------------
# Host-side input packing
# ---------------------------------------------------------------------------

def _adjacency_T(idx: np.ndarray, mask: np.ndarray) -> np.ndarray:
    """A^T[m, n] = sum_k mask[n, k] * [idx[n, k] == m], packed [128, NCH, N] bf16."""
    n_ids = np.broadcast_to(np.arange(N, dtype=np.int64)[:, None], idx.shape)
    flat = idx.astype(np.int64) * N + n_ids
    at = np.bincount(
        flat.ravel(), weights=mask.astype(np.float64).ravel(), minlength=N * N
    ).astype(np.float32).reshape(N, N)
    pad = np.zeros((NCH * CH, N), np.float32)
    pad[:N] = at
    packed = pad.reshape(NCH, CH, N).transpose(1, 0, 2)
    return np.ascontiguousarray(packed.astype(np.float16))


def make_in_maps(inputs: dict) -> list[dict]:
    node_hidden = np.asarray(inputs["node_hidden"], np.float32)
    cell = np.asarray(inputs["cell"], np.float32)
    w_in = np.asarray(inputs["w_in"], np.float32)
    w_out = np.asarray(inputs["w_out"], np.float32)
    u = (np.asarray(inputs["u_in"], np.float32)
         + np.asarray(inputs["u_out"], np.float32))
    b = np.asarray(inputs["b"], np.float32)
    in_mask = np.asarray(inputs["in_node_mask"], np.float32)
    out_mask = np.asarray(inputs["out_node_mask"], np.float32)
    in_idx = np.asarray(inputs["in_node_index"])
    out_idx = np.asarray(inputs["out_node_index"])
    assert int(np.asarray(inputs["layer_num"])) == LAYERS

    # stacked gate weights as lhsT [h, d]: u(4), w_in(4), w_out(4)
    w_stack = np.concatenate([u, w_in, w_out], axis=0)  # [12, H, H]
    w_pack = np.ascontiguousarray(w_stack.transpose(1, 0, 2))  # [128, 12, H]
    bt = np.ascontiguousarray(b.T)  # [128, 4]

    in_maps = []
    for bi in range(B):
        h = node_hidden[bi]                                   # [N, H]
        hpad = np.zeros((NCH * CH, H), np.float32)
        hpad[:N] = h
        h0n = np.ascontiguousarray(
            hpad.reshape(NCH, CH, H).transpose(1, 0, 2).astype(np.float16)
        )
        in_maps.append({
            "at_in": _adjacency_T(in_idx[bi], in_mask[bi]),
            "at_out": _adjacency_T(out_idx[bi], out_mask[bi]),
            "h0t": np.ascontiguousarray(h.T),
            "h0n": h0n,
            "c0t": np.ascontiguousarray(cell[bi].T),
            "w": w_pack,
            "bt": bt,
        })
    return in_maps


def unshard(results: list[dict]) -> np.ndarray:
    return np.stack(
        [np.ascontiguousarray(r["ht_out"].T) for r in results]
    ).astype(np.float32)


@functools.lru_cache(maxsize=1)
def _cached_nc() -> bass.Bass:
    return build_nc()


@functools.lru_cache(maxsize=1)
def _cached_exec():
    """Sharded PJRT callable over the 8 cores (mirrors run_bass_via_pjrt)."""
    import jax
    from jax.experimental.shard_map import shard_map
    from jax.sharding import Mesh, PartitionSpec
    from concourse import bass2jax

    nc = _cached_nc()
    bass2jax.install_neuronx_cc_hook()

    partition_name = (
        nc.partition_id_tensor.name if nc.partition_id_tensor else None
    )
    in_names: list[str] = []
    out_names: list[str] = []
    out_avals: list[jax.core.ShapedArray] = []
    zero_outs: list[np.ndarray] = []
    for alloc in nc.m.functions[0].allocations:
        if not isinstance(alloc, mybir.MemoryLocationSet):
            continue
        name = alloc.memorylocations[0].name
        if alloc.kind == "ExternalInput":
            if name == partition_name:
                continue
            in_names.append(name)
        elif alloc.kind == "ExternalOutput":
            out_names.append(name)
            shape = tuple(alloc.tensor_shape)
            dtype = mybir.dt.np(alloc.dtype)
            out_avals.append(jax.core.ShapedArray(shape, dtype))
            zero_outs.append(np.zeros(shape, dtype))
    n_params = len(in_names)
    n_outs = len(out_avals)
    all_names = in_names + out_names
    if partition_name is not None:
        all_names = all_names + [partition_name]
    donate = tuple(range(n_params, n_params + n_outs))

    def _body(*args):
        operands = list(args)
        if partition_name is not None:
            operands.append(bass2jax.partition_id_tensor())
        outs = bass2jax._bass_exec_p.bind(
            *operands,
            out_avals=tuple(out_avals),
            in_names=tuple(all_names),
            out_names=tuple(out_names),
            lowering_input_output_aliases=(),
            sim_require_finite=True,
            sim_require_nnan=True,
            nc=nc,
        )
        return tuple(outs)

    devices = jax.devices()[:NCORES]
    mesh = Mesh(np.asarray(devices), ("core",))
    in_specs = (PartitionSpec("core"),) * (n_params + n_outs)
    out_specs = (PartitionSpec("core"),) * n_outs
    sharded = jax.jit(
        shard_map(
            _body, mesh=mesh, in_specs=in_specs, out_specs=out_specs, check_rep=False
        ),
        donate_argnums=donate,
        keep_unused=True,
    )
    return sharded, mesh, in_names, out_names, out_avals, zero_outs


def _concat_inputs(in_maps: list[dict], in_names: list[str]) -> list[np.ndarray]:
    return [
        np.concatenate([np.asarray(in_maps[c][n]) for c in range(NCORES)], axis=0)
        for n in in_names
    ]


def _run(in_maps: list[dict]) -> list[dict]:
    sharded, mesh, in_names, out_names, out_avals, zero_outs = _cached_exec()
    concat_in = _concat_inputs(in_maps, in_names)
    concat_zeros = [
        np.zeros((NCORES * z.shape[0], *z.shape[1:]), z.dtype) for z in zero_outs
    ]
    out_arrs = sharded(*concat_in, *concat_zeros)
    return [
        {
            n: np.asarray(out_arrs[i]).reshape(NCORES, *out_avals[i].shape)[c]
            for i, n in enumerate(out_names)
        }
        for c in range(NCORES)
    ]


def kernel(**inputs) -> np.ndarray:
    in_maps = make_in_maps(inputs)
    return unshard(_run(in_maps))


def bench(inputs: dict, iters: int = 30) -> float:
    """Mean per-iteration device time (ns) over chained async executions."""
    import time
    import jax
    from jax.sharding import NamedSharding, PartitionSpec

    sharded, mesh, in_names, out_names, out_avals, zero_outs = _cached_exec()
    in_maps = make_in_maps(inputs)
    concat_in = _concat_inputs(in_maps, in_names)
    sh = NamedSharding(mesh, PartitionSpec("core"))
    dev_in = [jax.device_put(a, sh) for a in concat_in]
    zero_sets = [
        [
            jax.device_put(
                np.zeros((NCORES * z.shape[0], *z.shape[1:]), z.dtype), sh
            )
            for z in zero_outs
        ]
        for _ in range(iters + 1)
    ]
    # warmup
    outs = sharded(*dev_in, *zero_sets[-1])
    jax.block_until_ready(outs)
    t0 = time.time()
    for i in range(iters):
        outs = sharded(*dev_in, *zero_sets[i])
    jax.block_until_ready(outs)
    t1 = time.time()
    return (t1 - t0) / iters * 1e9
